# revision 5
# baseline (speedup 1.0000x reference)
"""Trainium2 Bass kernel for nn_PlanNotesProjection — v2 (tail-optimized).

Math (per batch b):
  own_f   = ownership[b].astype(f32)             # (K=32, S=4096)
  summed  = own_f @ emb[b]                       # (K, H=2048)
  counts  = clip(own_f.sum(-1), min=1)           # (K,)
  pooled  = summed / counts[:, None]
  proj    = pooled @ W + bias                    # (K, D=1024)
  out[b]  = LayerNorm(proj) * gamma + beta       # eps=1e-5

v2 changes vs v1 (all aimed at the post-stream tail, which the timeline
sim showed was 8.25us of the 39.7us total):
  1. Host-centered W: wP holds W' = W - rowmean(W), so every projection
     row has exactly zero mean by construction (sum_d v'_kd = pooled_k @
     rowsums(W') = 0; the fp8 re-quantization leaves a residual mean of
     ~0.04% of sigma_v). The LN epilogue needs no mean at all — just a
     sum of squares and a per-row scale.
  2. Parallel sumsq: bank0 via ACT activation(Square, accum_out);
     bank1 via DVE bn_stats (a DVE op may read only one PSUM input, so
     sumsq1 = 512*(var1+mean1^2)) — the two legs run concurrently.
  3. Parallel normalize (pure per-row scale): bank0 via DVE
     tensor_scalar_mul(rstd); bank1 via ACT activation(Identity,
     scale=rstd). Square/Sqrt/Identity all live in one ACT table
     (sqrt_and_others, pinned by an early dummy Sqrt) so no mid-kernel
     ACT table swap.
  4. Single [K,1024] store (one 625ns HWDGE desc-gen + one 650ns DGE
     launch instead of two serialized chains).
  5. Tail scheduling: proj(14) and proj(15) are emitted after pool(15)
     so proj(14) (whose operands are long ready) fills the PE while the
     DVE stage-copy of tile 15 round-trips; tile 14's halves swap rings
     so the slower Act-ring SEQ doesn't reorder the last arrivals.

Everything else (fp8 e3m4 streaming, host swizzles, two HWDGE rings,
depth-2 proj software pipeline, LN scale-invariance absorbing the
counts division and W_SCALE) is unchanged from v1 — see below.

Sharding: data-parallel over B across 8 cores (one batch per core);
W/b/gamma/beta replicated. Host swizzles make every device DMA fully
contiguous per partition:
  embP[p, (h*SC + c)*128 + j] = emb[c*128+p, h*128+j]   (fp8 e3m4)
  ownP[p, c*K + k]            = own[k, c*128+p]          (fp8 e3m4)
  wP[p, h*D + d]              = 64 * (W - rowmean(W))[h*128+p, d]  (fp8)
"""

import sys
from contextlib import ExitStack

import numpy as np

sys.path.insert(0, "/opt/trn_rl_repo")

import ml_dtypes

FP8E3 = ml_dtypes.float8_e3m4

B, K, S, H, D = 8, 32, 4096, 2048, 1024
LN_EPS = 1e-5
P = 128
SC = S // P    # 32 contraction chunks (S on partitions)
HC = H // P    # 16 h-tiles
DJ = 2        # psum column tiles for projection
BW = D // DJ  # 512 columns per projection bank
HB = SC // 2   # half an h-tile's chunks per DMA

TRACE = False
TRACE_TMPDIR = None
LAST_RESULT = None
_NC = None
_NC_KEY = None


def _prep_emb(emb_b):
    # (S, H) f32 -> (P, HC*SC*128) fp8 e3m4 with
    # embP[p, (h*SC+c)*128+j] = emb[c*128+p, h*128+j]
    return np.ascontiguousarray(
        emb_b.astype(FP8E3).reshape(SC, P, HC, P).transpose(1, 2, 0, 3)
        .reshape(P, HC * SC * P))


def _prep_own(own_b):
    # (K, S) bool -> (P, SC*K) fp8 with ownP[p, c*K+k] = own[k, c*128+p]
    return np.ascontiguousarray(
        own_b.T.astype(FP8E3).reshape(SC, P, K).transpose(1, 0, 2).reshape(P, SC * K))


W_SCALE = 64.0  # lifts W ~N(0, 1/sqrt(2048)) into e3m4's normal range


def _prep_w(wmat):
    # (H, D) f32 -> (P, HC*D) fp8 e3m4. W is row-CENTERED on the host
    # (W' = W - rowmean(W)) before scaling: every projection row then has
    # exactly zero mean by construction (sum_d v'_kd = pooled_k @ rowsums
    # = 0), so the LayerNorm epilogue needs no mean at all — only the sum
    # of squares. The fp8 quantization of W' leaves a residual row-sum of
    # ~0.04% of sigma_v: far below the error budget. The global W_SCALE is
    # absorbed by LayerNorm scale invariance (eps_k picks up W_SCALE^2).
    wc = wmat - wmat.mean(axis=1, keepdims=True)
    return np.ascontiguousarray(
        (wc * W_SCALE).astype(FP8E3).reshape(HC, P, D).transpose(1, 0, 2)
        .reshape(P, HC * D))


def _build_nc(repeats=1, has_bias=False, has_gamma=False, has_beta=False):
    import concourse.bass as bass
    import concourse.tile as tile
    from concourse import mybir
    from concourse.bacc import Bacc

    FP32 = mybir.dt.float32
    BF = mybir.dt.bfloat16
    F8 = mybir.dt.float8e3

    nc = Bacc("TRN2", target_bir_lowering=False)
    embP = nc.declare_dram_parameter("embP", [P, HC * SC * P], F8, False)
    ownP = nc.declare_dram_parameter("ownP", [P, SC * K], F8, False)
    wP = nc.declare_dram_parameter("wP", [P, HC * D], F8, False)
    bvec = nc.declare_dram_parameter("bvec", [D], FP32, False)
    gamma = nc.declare_dram_parameter("gamma", [D], FP32, False)
    beta = nc.declare_dram_parameter("beta", [D], FP32, False)
    out = nc.declare_dram_parameter("out", [K, D], FP32, True)

    with ExitStack() as ctx:
        tc = ctx.enter_context(tile.TileContext(nc))

        own_pool = ctx.enter_context(tc.tile_pool(name="own", bufs=1))
        w_pool = ctx.enter_context(tc.tile_pool(name="w", bufs=1))
        # Pool-recycled whole-h-tile buffers: consumer-pull pacing keeps the
        # DMA semaphore lanes unambiguous.
        emb_pool = ctx.enter_context(tc.tile_pool(name="emb", bufs=16))
        ones_pool = ctx.enter_context(tc.tile_pool(name="ones", bufs=2))
        cnt_pool = ctx.enter_context(tc.tile_pool(name="cnt", bufs=4))
        st_pool = ctx.enter_context(tc.tile_pool(name="st", bufs=3))
        bc_pool = ctx.enter_context(tc.tile_pool(name="bc", bufs=1))
        x_pool = ctx.enter_context(tc.tile_pool(name="x", bufs=3))
        stats_pool = ctx.enter_context(tc.tile_pool(name="stats", bufs=1))
        # distinct slots for the many small [K,1] epilogue vectors
        mv_pool = ctx.enter_context(tc.tile_pool(name="mv", bufs=12))

        psum_sum = ctx.enter_context(tc.tile_pool(name="psum_sum", bufs=2, space="PSUM"))
        psum_proj = ctx.enter_context(tc.tile_pool(name="psum_proj", bufs=1, space="PSUM"))
        psum_cnt = ctx.enter_context(tc.tile_pool(name="psum_cnt", bufs=1, space="PSUM"))

        def body():
            # own rides the SWDGE (gpsimd) queue: its desc-gen runs off
            # the shared HWDGE sequencer, so the emb stream's first HWDGE
            # descriptor (and the whole work-conserving pipe) starts
            # ~625ns earlier than if own led a ring.
            own_sb = own_pool.tile([P, SC, K], F8)
            nc.gpsimd.dma_start(out=own_sb[:], in_=ownP[:, :])

            # W quarter 0 on the scalar ring up-front; quarters 1..3
            # interleave with the emb stream.
            w_sb = w_pool.tile([P, HC, D], F8)
            nc.scalar.dma_start(w_sb[:, 0:4, :], wP[:, 0:4 * D])

            ones = ones_pool.tile([P, 1], F8)
            nc.vector.memset(ones[:], 1.0)

            # Dummy Sqrt: the act-table pass greedily picks the first table
            # containing the FIRST activation func seen; Sqrt's first match
            # (set 3, sqrt_and_others) also contains Square and Identity, so
            # this pins ONE table load, hoisted to kernel start and hidden
            # under the emb stream. Without it, Square first-matches set 0
            # and the Sqrt forces a 1283ns mid-epilogue table swap.
            dum = ones_pool.tile([1, 1], FP32)
            nc.vector.memset(dum[:], 1.0)
            nc.scalar.activation(out=dum[:], in_=dum[:],
                                 func=mybir.ActivationFunctionType.Sqrt,
                                 bias=0.0, scale=1.0, alpha=0.0)

            def bcast(vec):
                t = bc_pool.tile([K, D], FP32, name=f"bc_{vec.name}")
                ap = vec[:]
                bc_ap = bass.AP(tensor=ap.tensor, offset=ap.offset, ap=[[0, K]] + list(ap.ap))
                nc.gpsimd.dma_start(out=t[:], in_=bc_ap)
                return t

            bias_bc = bcast(bvec) if has_bias else None
            gam_bc = bcast(gamma) if has_gamma else None
            bet_bc = bcast(beta) if has_beta else None

            cnt_ps = psum_cnt.tile([K, 1], FP32)
            proj_ps = [psum_proj.tile([K, BW], FP32, name=f"proj_ps{jj}") for jj in range(DJ)]
            pipe = []  # (h, st_sb) awaiting their proj matmuls

            def proj_step(hh, st, stop):
                for jj in range(DJ):
                    nc.tensor.matmul(proj_ps[jj][:], st[:],
                                     w_sb[:, hh, jj * BW:(jj + 1) * BW],
                                     start=(hh == 0), stop=stop)

            for h in range(HC):
                # Each h-tile streams as two halves on the two HWDGE rings
                # (SP via nc.sync, Activation ring via nc.scalar). The LAST
                # FOUR pieces (etB14, etA15, etB15x, etB15y) all ride ring A:
                # a single ring's pieces arrive strictly in issue order, so
                # the pooling emission order is guaranteed to match data
                # arrival at the stream tail (cross-ring arbitration was
                # measured inverting etB14/etA15, stalling the in-order PE
                # queue ~700ns). Tile 14's A-half compensates on ring B. The
                # final piece is only 4 chunks so just 52ns of pooling trails
                # the last DMA's 900ns completion-semaphore propagation.
                base = h * SC * P
                last = h == HC - 1
                engA = nc.scalar if h == HC - 2 else nc.sync
                etA = emb_pool.tile([P, HB, P], F8)
                if last:
                    with tc.high_priority():
                        engA.dma_start(etA[:], embP[:, base:base + HB * P])
                else:
                    engA.dma_start(etA[:], embP[:, base:base + HB * P])
                # Tile 15's B half goes on ring A right behind its A half:
                # same-ring FIFO guarantees A lands first, so pool(15)'s
                # first 16 chunks run during the final DMA and only the B
                # half's 16 chunks (208ns) trail the last 900ns semaphore.
                engB = nc.sync if h >= HC - 2 else nc.scalar
                etB = emb_pool.tile([P, HB, P], F8)
                if last:
                    with tc.high_priority():
                        engB.dma_start(etB[:], embP[:, base + HB * P:base + SC * P])
                else:
                    engB.dma_start(etB[:], embP[:, base + HB * P:base + SC * P])
                if h in (2, 5, 8):
                    q = {2: 1, 5: 2, 8: 3}[h]
                    weng = nc.scalar if q == 2 else nc.sync
                    weng.dma_start(w_sb[:, 4 * q:4 * (q + 1), :],
                                   wP[:, 4 * q * D:4 * (q + 1) * D])

                st_ps = psum_sum.tile([P, 512], FP32)
                for c in range(HB):
                    nc.tensor.matmul(st_ps[:, 0:K], etA[:, c, :], own_sb[:, c, :],
                                     start=(c == 0), stop=False)
                for c in range(HB, SC):
                    nc.tensor.matmul(st_ps[:, 0:K], etB[:, c - HB, :], own_sb[:, c, :],
                                     start=False, stop=(c == SC - 1))
                if h == 0:
                    # counts[k] = sum_s own[k, s] — fills PE slack.
                    for c in range(SC):
                        nc.tensor.matmul(cnt_ps[:], own_sb[:, c, :], ones[:],
                                         start=(c == 0), stop=(c == SC - 1))
                # Stage copy on the Vector engine (Scalar queue stays pure
                # DMA issues mid-stream).
                st_sb = st_pool.tile([P, K], BF)
                nc.vector.tensor_copy(out=st_sb[:], in_=st_ps[:, 0:K])
                pipe.append((h, st_sb))
                if len(pipe) > 1 and not last:
                    hh, st = pipe.pop(0)
                    proj_step(hh, st, stop=False)
            # pipe holds (14, 15): proj(14) lands right after pool(15)'s
            # last chunk (its operands are long ready) and overlaps the DVE
            # stage copy of tile 15; proj(15) follows.
            hh, st = pipe.pop(0)
            proj_step(hh, st, stop=False)
            hh, st = pipe.pop(0)
            proj_step(hh, st, stop=True)

            cnt_sb = cnt_pool.tile([K, 1], FP32)
            nc.vector.tensor_scalar_max(out=cnt_sb[:], in0=cnt_ps[:], scalar1=1.0)

            if has_bias:
                # General path: pooled = summed/counts materialized before
                # the bias add; plain-eps LayerNorm via bn_stats.
                cnt64 = cnt_pool.tile([K, 1], FP32)
                nc.vector.tensor_scalar_mul(out=cnt64[:], in0=cnt_sb[:], scalar1=W_SCALE)
                inv_sb = cnt_pool.tile([K, 1], FP32)
                nc.vector.reciprocal(out=inv_sb[:], in_=cnt64[:])
                eps_k = cnt_pool.tile([K, 1], FP32)
                nc.vector.memset(eps_k[:], LN_EPS)
                x = x_pool.tile([K, D], FP32)
                for jj in range(DJ):
                    nc.vector.tensor_scalar_mul(
                        out=x[:, jj * BW:(jj + 1) * BW], in0=proj_ps[jj][:], scalar1=inv_sb[:],
                    )
                nc.vector.tensor_add(out=x[:], in0=x[:], in1=bias_bc[:])
                src = [x[:, jj * BW:(jj + 1) * BW] for jj in range(DJ)]

                stats = stats_pool.tile([K, DJ, nc.vector.BN_STATS_DIM], FP32)
                for g in range(DJ):
                    nc.vector.bn_stats(out=stats[:, g, :], in_=src[g])
                mv = mv_pool.tile([K, nc.vector.BN_AGGR_DIM], FP32)
                nc.vector.bn_aggr(out=mv[:], in_=stats[:])
                rstd = mv_pool.tile([K, 1], FP32)
                nc.scalar.activation(
                    out=rstd[:], in_=mv[:, 1:2],
                    func=mybir.ActivationFunctionType.Sqrt, bias=eps_k[:], scale=1.0, alpha=0.0,
                )
                nc.vector.reciprocal(out=rstd[:], in_=rstd[:])
                outt = x_pool.tile([K, D], FP32)
                for jj in range(DJ):
                    half = outt[:, jj * BW:(jj + 1) * BW]
                    nc.vector.tensor_scalar(
                        out=half, in0=src[jj], scalar1=mv[:, 0:1], scalar2=rstd[:],
                        op0=mybir.AluOpType.subtract, op1=mybir.AluOpType.mult,
                    )
                    if has_gamma:
                        nc.vector.tensor_mul(out=half, in0=half, in1=gam_bc[:, jj * BW:(jj + 1) * BW])
                    if has_beta:
                        nc.vector.tensor_add(out=half, in0=half, in1=bet_bc[:, jj * BW:(jj + 1) * BW])
                nc.sync.dma_start(out[:, :], outt[:])
                return

            # Fast path (bias==0, gamma==1, beta==0): LN scale invariance
            # normalizes the raw v = W_SCALE*summed@Wc directly, and the
            # host-centered W makes every row of v exactly zero-mean, so
            # the epilogue is just a sum-of-squares and a scale:
            #   rstd = 1/sqrt(sumsq/D + eps_k),  eps_k = LN_EPS*(c*W_SCALE)^2
            #   out  = v * rstd
            cnt2 = cnt_pool.tile([K, 1], FP32)
            nc.vector.tensor_mul(out=cnt2[:], in0=cnt_sb[:], in1=cnt_sb[:])
            eps_k = cnt_pool.tile([K, 1], FP32)
            nc.vector.tensor_scalar_mul(out=eps_k[:], in0=cnt2[:],
                                        scalar1=LN_EPS * W_SCALE * W_SCALE)

            # Stats split across engines and banks (ACT and DVE may touch
            # PSUM concurrently only on DIFFERENT banks): ACT Square-
            # accumulates bank0 (the slower leg, started first) while DVE
            # bn_stats bank1 (a DVE op may read only ONE input from PSUM,
            # ruling out tensor_tensor_reduce(x,x)); bank1's sum of squares
            # is recovered as 512*(var1 + mean1^2).
            sumsq0 = mv_pool.tile([K, 1], FP32)
            scr0 = x_pool.tile([K, BW], FP32)
            nc.scalar.activation(
                out=scr0[:], in_=proj_ps[0][:],
                func=mybir.ActivationFunctionType.Square,
                bias=0.0, scale=1.0, alpha=0.0, accum_out=sumsq0[:],
            )
            stats1 = stats_pool.tile([K, 1, nc.vector.BN_STATS_DIM], FP32)
            nc.vector.bn_stats(out=stats1[:, 0, :], in_=proj_ps[1][:])
            mv1 = mv_pool.tile([K, nc.vector.BN_AGGR_DIM], FP32)
            nc.vector.bn_aggr(out=mv1[:], in_=stats1[:])
            m1sq = mv_pool.tile([K, 1], FP32)
            nc.vector.tensor_mul(out=m1sq[:], in0=mv1[:, 0:1], in1=mv1[:, 0:1])
            h1 = mv_pool.tile([K, 1], FP32)
            nc.vector.tensor_add(out=h1[:], in0=mv1[:, 1:2], in1=m1sq[:])
            # tk = 0.5*h1 + eps_k; the ACT-side sum of squares is folded
            # into the Sqrt as in*scale: rstd_pre = Sqrt(sumsq0/D + tk).
            tk = mv_pool.tile([K, 1], FP32)
            nc.vector.tensor_scalar(
                out=tk[:], in0=h1[:], scalar1=0.5, scalar2=eps_k[:],
                op0=mybir.AluOpType.mult, op1=mybir.AluOpType.add,
            )
            rstd = mv_pool.tile([K, 1], FP32)
            nc.scalar.activation(
                out=rstd[:], in_=sumsq0[:],
                func=mybir.ActivationFunctionType.Sqrt,
                bias=tk[:], scale=1.0 / D, alpha=0.0,
            )
            nc.vector.reciprocal(out=rstd[:], in_=rstd[:])
            # Normalize is a pure per-row scale: DVE bank0, ACT bank1 on
            # distinct PSUM banks concurrently.
            outt = x_pool.tile([K, D], FP32)
            nc.vector.tensor_scalar_mul(
                out=outt[:, 0:BW], in0=proj_ps[0][:], scalar1=rstd[:],
            )
            nc.scalar.activation(
                out=outt[:, BW:2 * BW], in_=proj_ps[1][:],
                func=mybir.ActivationFunctionType.Identity,
                bias=0.0, scale=rstd[:], alpha=0.0,
            )
            nc.sync.dma_start(out[:, :], outt[:])

        for _ in range(repeats):
            body()

    nc.finalize()
    return nc


def kernel(**inputs: np.ndarray) -> np.ndarray:
    global _NC, _NC_KEY, LAST_RESULT
    from concourse.bass_utils import run_bass_kernel_spmd

    emb = np.asarray(inputs["plan_embeddings"], dtype=np.float32)
    own = np.asarray(inputs["ownership"])
    wmat = np.ascontiguousarray(np.asarray(inputs["W"], dtype=np.float32))
    bv = np.ascontiguousarray(np.asarray(inputs["b"], dtype=np.float32))
    ga = np.ascontiguousarray(np.asarray(inputs["gamma"], dtype=np.float32))
    be = np.ascontiguousarray(np.asarray(inputs["beta"], dtype=np.float32))

    key = (bool(np.any(bv != 0.0)), bool(np.any(ga != 1.0)), bool(np.any(be != 0.0)))
    if _NC is None or _NC_KEY != key:
        _NC = _build_nc(has_bias=key[0], has_gamma=key[1], has_beta=key[2])
        _NC_KEY = key

    wP = _prep_w(wmat)
    in_maps = []
    for i in range(B):
        in_maps.append({
            "embP": _prep_emb(emb[i]),
            "ownP": _prep_own(own[i]),
            "wP": wP,
            "bvec": bv,
            "gamma": ga,
            "beta": be,
        })
    res = run_bass_kernel_spmd(_NC, in_maps, core_ids=list(range(B)), trace=TRACE,
                               tmpdir=TRACE_TMPDIR)
    LAST_RESULT = res
    return np.stack([np.asarray(res.results[i]["out"]) for i in range(B)], axis=0).astype(np.float32)
